# revision 35
# baseline (speedup 1.0000x reference)
"""Trainium2 Bass kernel for LongformerForSentenceClassification
(segment-mean pooling over sep-delimited sentences + 3-layer MLP head).

Strategy: data-parallel over the batch dim B=8 across the 8 NeuronCores —
one batch row per core.  The data-dependent segment pooling is expressed as
a dense matmul sent = A @ h, where the (tiny) assignment matrix A
[MAX_SENT, S] is built on the host from input_ids with exactly the
reference semantics (weights, truncation, count normalization).  All heavy
compute runs on-device in fp16 (fp32 PSUM accumulation):

    pooling:  sent[64, 768]   = A[64, 4096] @ h[4096, 768]
    MLP1:     x1[64, 4096]    = gelu(sent @ W1 + b1)
    MLP2:     x2[64, 256]     = gelu(x1 @ W2 + b2)
    MLP3:     logits[64, 2]   = x2 @ W3 + b3

Between layers the activation must be re-laid-out feature-major to serve
as the next matmul's stationary operand (lhsT); those transposes go
through the DMA x-bar (fp16, SBUF->SBUF).  Biases are folded into the
matmul accumulation as K=1 matmuls with a ones-vector lhsT, and skipped
entirely when the host sees an all-zero bias.
"""

import numpy as np

import concourse.bass as bass
import concourse.mybir as mybir
import concourse.tile as tile
from concourse.masks import make_identity
from concourse.vector_clock import ScopedClock
from concourse.bass_utils import run_bass_kernel_spmd

SEP = 2
B, S, H = 8, 4096, 768
MAX_SENT = 64
F1, F2, NCLS = 4096, 256, 2
N_CORES = 8

KS = S // 128          # 32 k-chunks over tokens
KH = H // 128          # 6  k-chunks over hidden dim
KF1 = F1 // 128        # 32 k-chunks over F1
KF2 = F2 // 128        # 2  k-chunks over F2
N1 = F1 // 512         # 8  n-chunks of MLP1 output
HJ = 4                 # h tile granularity: 4 k-chunks per DMA tile
FP16 = mybir.dt.float16
F32 = mybir.dt.float32
GELU = mybir.ActivationFunctionType.Gelu

# exec-time metadata from the most recent kernel() call (filled when
# BASS_TRACE=1); harmless extra attribute for test harnesses.
LAST_META = {}


class SplitDrainTileContext(tile.TileContext):
    """The walrus build in this container only accepts a single sync-wait
    on the kernel-tail Drain instruction; emit the global-clock waits as
    individual wait_ge instructions instead of stacking them on the drain."""

    def _drain_and_barrier(self, tick_clock, wait_clock):
        nc = self.nc
        probe = nc.sync.nop(nofuse=True)
        wait_clock.add_sem_waits(
            probe.ins, ScopedClock({None: tick_clock.global_clock})
        )
        si = probe.ins.sync_info
        waits = list(si.on_wait) if si is not None and si.on_wait else []
        if si is not None and si.on_wait:
            si.on_wait.clear()
        sem_by_num = {s.num: s for s in self.sems.allocated().values()}
        for w in waits:
            assert w.wait_mode == "sem-ge-imm", w
            nc.sync.wait_ge(sem_by_num[w.id], w.wait_value)
        nc.sync.drain()
        nc.all_engine_barrier()
        popped = nc._tile_sem_poison_stack.pop()
        assert popped is self._sem_poison
        nc.clear_and_free_semaphores(list(self.sems.allocated().values()))
        nc.all_engine_barrier()


def _split_multi_waits(nc) -> None:
    """The walrus build here rejects instructions carrying more than one
    sync-wait ("Too many sync wait commands").  Hoist all but the last wait
    of every instruction onto dedicated same-engine NoOps placed directly
    before it — semantically identical (the engine blocks on each wait in
    order before executing the instruction)."""
    for bb in nc.m.functions[0].blocks:
        insts = bb.instructions
        i = 0
        while i < len(insts):
            inst = insts[i]
            si = inst.sync_info
            if si is not None and si.on_wait and len(si.on_wait) > 1:
                extra = list(si.on_wait[:-1])
                keep = si.on_wait[-1]
                si.on_wait.clear()
                si.on_wait.append(keep)
                for j, w in enumerate(extra):
                    nop = mybir.InstNoOp(
                        name=nc.get_next_instruction_name(),
                        sync_info=mybir.SyncInfo(on_wait=[w], on_update=[]),
                        bass_nofuse=True,
                        engine=inst.engine,
                    )
                    nc.register_instruction(nop)
                    insts.insert(i + j, nop)
                i += len(extra)
            i += 1


def _pool_meta(ids: np.ndarray):
    """[B, S] token ids -> (seg_eff [B, S] int32, inv_cnt [B, MAX_SENT] f32)
    matching the reference segment-mean semantics exactly.  seg_eff is the
    clamped segment id, with weight-excluded tokens pointed at the dump
    bucket MAX_SENT; inv_cnt is 1/token-count per sentence (empty -> the
    sums are zero anyway, so the scale value there is irrelevant)."""
    ids = np.asarray(ids)
    sep = ids == SEP
    sep_i = sep.astype(np.int64)
    seg = np.cumsum(sep_i, axis=1) - sep_i          # exclusive cumsum
    n_sep = sep_i.sum(axis=1)                       # [B]
    first_sep = np.argmax(sep, axis=1)              # 0 if no sep at all
    pos = np.arange(ids.shape[1])
    # the first sep belongs to sentence 0; later seps are excluded
    w = np.where(sep, pos[None, :] == first_sep[:, None], True)
    # exclude last token of the trailing (post-last-sep) segment
    w &= ~(
        (pos[None, :] == ids.shape[1] - 1)
        & (seg == n_sep[:, None])
        & (n_sep[:, None] > 0)
    )
    seg_c = np.minimum(seg, MAX_SENT)               # overflow -> dump bucket
    seg_eff = np.where(w, seg_c, MAX_SENT).astype(np.int32)
    cnt = (seg_eff[:, None, :] == np.arange(MAX_SENT)[None, :, None]).sum(axis=2)
    inv_cnt = (1.0 / np.maximum(cnt, 1)).astype(np.float32)
    return seg_eff, inv_cnt


_BUILD_CACHE = {}


def _build(with_b1: bool, with_b2: bool, b3_vals: tuple):
    key = (with_b1, with_b2, b3_vals)
    if key in _BUILD_CACHE:
        return _BUILD_CACHE[key]

    nc = bass.Bass()
    h_d = nc.declare_dram_parameter("h", [128, KS * H], FP16, isOutput=False)
    seg_d = nc.declare_dram_parameter("seg", [128, KS], F32, isOutput=False)
    invc_d = nc.declare_dram_parameter("invc", [MAX_SENT, 1], F32, isOutput=False)
    w1_d = nc.declare_dram_parameter("w1", [128, N1 * KH * 512], FP16, isOutput=False)
    w2_d = nc.declare_dram_parameter("w2", [128, KF1 * F2], FP16, isOutput=False)
    w3_d = nc.declare_dram_parameter(
        "w3", [MAX_SENT, NCLS, F2], FP16, isOutput=False
    )
    b1_d = b2_d = None
    if with_b1:
        b1_d = nc.declare_dram_parameter("b1", [1, F1], FP16, isOutput=False)
    if with_b2:
        b2_d = nc.declare_dram_parameter("b2", [1, F2], FP16, isOutput=False)
    out_d = nc.declare_dram_parameter("out", [MAX_SENT, NCLS], F32, isOutput=True)

    with SplitDrainTileContext(nc) as tc:
        with (
            tc.tile_pool(name="wpool", bufs=1) as wpool,
            tc.tile_pool(name="apool", bufs=1) as apool,
            tc.tile_pool(name="psacc", bufs=1, space="PSUM") as psacc,
            tc.tile_pool(name="ps1", bufs=2, space="PSUM") as ps1pool,
            tc.tile_pool(name="psT", bufs=2, space="PSUM") as psTpool,
        ):
            # [64, 64] identity: rhs operand for PE-mode transposes of
            # [64, 128] activation slices (DMA-xbar transposes would
            # serialize behind the big weight-load DMA stream)
            ident = wpool.tile([MAX_SENT, MAX_SENT], FP16, tag="ident")
            make_identity(nc, ident[:])

            def pe_transpose(dst, src):
                """dst [128, 64] (sbuf) = src [64, 128] (sbuf) transposed."""
                psT = psTpool.tile([128, MAX_SENT], FP16, tag="psT")
                nc.tensor.transpose(psT[:], src, ident[:])
                nc.vector.tensor_copy(out=dst, in_=psT[:])

            # ---- input loads, in consumption order ----
            # build the pooling assignment matrix on-device: at[p, k, m] =
            # (seg_id[token k*128+p] == m), from a 16 KB seg-id tensor
            # (weight-excluded tokens are pre-pointed at the dump id 64 on
            # the host; 1/count normalization is applied at PSUM eviction)
            h_sb = []
            for j in range(KS // HJ):
                t = wpool.tile([128, HJ, H], FP16, tag=f"h{j}")
                nc.sync.dma_start(
                    out=t[:],
                    in_=h_d[:, j * HJ * H : (j + 1) * HJ * H].rearrange(
                        "p (k h) -> p k h", k=HJ
                    ),
                )
                h_sb.append(t)
            seg_sb = wpool.tile([128, KS], F32, tag="seg")
            nc.sync.dma_start(out=seg_sb[:], in_=seg_d[:])
            invc_sb = wpool.tile([MAX_SENT, 1], F32, tag="invc")
            nc.sync.dma_start(out=invc_sb[:], in_=invc_d[:])
            iota_sb = wpool.tile([128, MAX_SENT], F32, tag="iota")
            nc.gpsimd.iota(iota_sb[:], pattern=[[1, MAX_SENT]], base=0,
                           channel_multiplier=0,
                           allow_small_or_imprecise_dtypes=True)
            at_sb = wpool.tile([128, KS, MAX_SENT], FP16, tag="at")
            for k in range(KS):
                nc.vector.tensor_scalar(
                    at_sb[:, k, :], iota_sb[:], seg_sb[:, k : k + 1], None,
                    op0=mybir.AluOpType.is_equal,
                )
            # w3 (tiny, broadcast layout for the DVE/ACT classifier) early
            w3_sb = wpool.tile([MAX_SENT, NCLS, F2], FP16, tag="w3")
            nc.sync.dma_start(out=w3_sb[:], in_=w3_d[:])
            # w1 tile n split into two k-halves so chunk n's first matmuls
            # start half a tile-transfer earlier (shortens the tail chain
            # behind the final w1 bytes)
            w1_sb = []
            for n in range(N1):
                t = wpool.tile([128, KH, 512], FP16, tag=f"w1{n}")
                for half in range(2):
                    k0, k1 = (0, KH // 2) if half == 0 else (KH // 2, KH)
                    nc.sync.dma_start(
                        out=t[:, k0:k1, :],
                        in_=w1_d[
                            :, (n * KH + k0) * 512 : (n * KH + k1) * 512
                        ].rearrange("p (k n) -> p k n", k=k1 - k0),
                    )
                w1_sb.append(t)
            # w2 in quarters: the last bytes of the load stream gate only
            # 8 of MLP2's 32 matmuls
            w2_sb = wpool.tile([128, KF1, F2], FP16, tag="w2")
            w2_pieces = [(0, 8), (8, 16), (16, 24), (24, 28), (28, 32)]
            for k0, k1 in w2_pieces:
                nc.sync.dma_start(
                    out=w2_sb[:, k0:k1, :],
                    in_=w2_d[:, k0 * F2 : k1 * F2].rearrange(
                        "p (k n) -> p k n", k=k1 - k0
                    ),
                )
            ones_sb = b1_sb = b2_sb = None
            if with_b1 or with_b2:
                ones_sb = wpool.tile([1, MAX_SENT], FP16, tag="ones")
                nc.vector.memset(ones_sb[:], 1.0)
            if with_b1:
                b1_sb = wpool.tile([1, F1], FP16, tag="b1")
                nc.sync.dma_start(out=b1_sb[:], in_=b1_d[:])
            if with_b2:
                b2_sb = wpool.tile([1, F2], FP16, tag="b2")
                nc.sync.dma_start(out=b2_sb[:], in_=b2_d[:])

            # ---- pooling: sent = A @ h  -> psum [64, 768] ----
            ps_sent = psacc.tile([MAX_SENT, H], F32, tag="ps_sent")
            for n0, nsz in ((0, 512), (512, 256)):
                for k in range(KS):
                    nc.tensor.matmul(
                        ps_sent[:, n0 : n0 + nsz],
                        lhsT=at_sb[:, k, :],
                        rhs=h_sb[k // HJ][:, k % HJ, n0 : n0 + nsz],
                        start=(k == 0),
                        stop=(k == KS - 1),
                    )
            sent_sb = apool.tile([MAX_SENT, H], FP16, tag="sent")
            nc.scalar.activation(
                sent_sb[:], ps_sent[:], mybir.ActivationFunctionType.Copy,
                bias=0.0, scale=invc_sb[:],
            )
            sentT = apool.tile([128, KH, MAX_SENT], FP16, tag="sentT")
            for k in range(KH):
                pe_transpose(sentT[:, k, :], sent_sb[:, k * 128 : (k + 1) * 128])

            # ---- MLP1: x1 = gelu(sent @ W1 + b1), chunked by 512 cols ----
            x1T = []
            for n in range(N1):
                ps = ps1pool.tile([MAX_SENT, 512], F32, tag="ps_x1")
                for k in range(KH):
                    nc.tensor.matmul(
                        ps[:],
                        lhsT=sentT[:, k, :],
                        rhs=w1_sb[n][:, k, :],
                        start=(k == 0),
                        stop=(k == KH - 1 and not with_b1),
                    )
                if with_b1:
                    nc.tensor.matmul(
                        ps[:],
                        lhsT=ones_sb[:, :],
                        rhs=b1_sb[:, n * 512 : (n + 1) * 512],
                        start=False,
                        stop=True,
                    )
                x1c = apool.tile([MAX_SENT, 512], FP16, tag=f"x1c{n}")
                nc.scalar.activation(x1c[:], ps[:], GELU)
                t = apool.tile([128, HJ, MAX_SENT], FP16, tag=f"x1T{n}")
                for c in range(HJ):
                    pe_transpose(t[:, c, :], x1c[:, c * 128 : (c + 1) * 128])
                x1T.append(t)

            # ---- MLP2: x2 = gelu(x1 @ W2 + b2) ----
            ps2 = psacc.tile([MAX_SENT, F2], F32, tag="ps_x2")
            for k in range(KF1):
                nc.tensor.matmul(
                    ps2[:],
                    lhsT=x1T[k // HJ][:, k % HJ, :],
                    rhs=w2_sb[:, k, :],
                    start=(k == 0),
                    stop=(k == KF1 - 1 and not with_b2),
                )
            if with_b2:
                nc.tensor.matmul(
                    ps2[:], lhsT=ones_sb[:, :], rhs=b2_sb[:, :], start=False, stop=True
                )
            x2_sb = apool.tile([MAX_SENT, F2], FP16, tag="x2")
            nc.scalar.activation(x2_sb[:], ps2[:], GELU)

            # ---- MLP3: logits[t, c] = sum_g x2[t, g] * W3[g, c] + b3[c] ----
            # tiny contraction (256 -> 2): one DVE multiply+reduce per class
            # against a host-broadcast W3, with b3[c] baked as the reduce
            # init — avoids transposing x2, keeps the tail chain short
            out_sb = apool.tile([MAX_SENT, NCLS], F32, tag="outsb")
            for c in range(NCLS):
                tmp = apool.tile([MAX_SENT, F2], FP16, tag=f"mlp3tmp{c}")
                nc.vector.tensor_mul(tmp[:], x2_sb[:], w3_sb[:, c, :])
                nc.vector.tensor_reduce(
                    out_sb[:, c : c + 1],
                    tmp[:],
                    axis=mybir.AxisListType.X,
                    op=mybir.AluOpType.add,
                )
            if any(v != 0.0 for v in b3_vals):
                for c in range(NCLS):
                    nc.vector.tensor_scalar_add(
                        out_sb[:, c : c + 1], out_sb[:, c : c + 1], float(b3_vals[c])
                    )
            nc.sync.dma_start(out=out_d[:], in_=out_sb[:])

    _split_multi_waits(nc)
    _BUILD_CACHE[key] = nc
    return nc


def kernel(hidden, input_ids, W1, b1, W2, b2, W3, b3):
    hidden = np.asarray(hidden, dtype=np.float32)
    W1 = np.asarray(W1, dtype=np.float32)
    W2 = np.asarray(W2, dtype=np.float32)
    W3 = np.asarray(W3, dtype=np.float32)
    b1 = np.asarray(b1, dtype=np.float32)
    b2 = np.asarray(b2, dtype=np.float32)
    b3 = np.asarray(b3, dtype=np.float32)

    seg_eff, inv_cnt = _pool_meta(input_ids)            # [B, S], [B, 64]

    # pack per-core operands [128 partitions, free] so every DMA line is
    # fully contiguous.  token t = k*128 + p; feature f = k*128 + p.
    h16 = hidden.astype(np.float16)
    h_pack = np.ascontiguousarray(
        h16.reshape(B, KS, 128, H).transpose(0, 2, 1, 3)
    ).reshape(B, 128, KS * H)
    seg_pack = np.ascontiguousarray(
        seg_eff.astype(np.float32).reshape(B, KS, 128).transpose(0, 2, 1)
    )                                                   # [B, 128, KS]
    invc_pack = inv_cnt.reshape(B, MAX_SENT, 1)
    w1_pack = np.ascontiguousarray(
        W1.astype(np.float16).reshape(KH, 128, N1, 512).transpose(1, 2, 0, 3)
    ).reshape(128, N1 * KH * 512)
    w2_pack = np.ascontiguousarray(
        W2.astype(np.float16).reshape(KF1, 128, F2).transpose(1, 0, 2)
    ).reshape(128, KF1 * F2)
    # W3 broadcast across the 64 sentence partitions for the DVE classifier
    w3_pack = np.ascontiguousarray(
        np.broadcast_to(
            W3.T.astype(np.float16).reshape(1, NCLS, F2), (MAX_SENT, NCLS, F2)
        )
    )

    with_b1 = bool(np.any(b1))
    with_b2 = bool(np.any(b2))
    nc = _build(with_b1, with_b2, tuple(float(v) for v in b3))

    in_maps = []
    for c in range(N_CORES):
        m = {
            "h": h_pack[c],
            "seg": seg_pack[c],
            "invc": invc_pack[c],
            "w1": w1_pack,
            "w2": w2_pack,
            "w3": w3_pack,
        }
        if with_b1:
            m["b1"] = b1.astype(np.float16).reshape(1, F1)
        if with_b2:
            m["b2"] = b2.astype(np.float16).reshape(1, F2)
        in_maps.append(m)

    res = run_bass_kernel_spmd(nc, in_maps, list(range(N_CORES)))
    LAST_META.clear()
    LAST_META["exec_time_ns"] = res.exec_time_ns
    LAST_META["mean_exec_time_ns"] = res.mean_exec_time_ns
    if res.instructions_and_trace is not None:
        LAST_META["trace"] = res.instructions_and_trace[1]

    return np.stack([res.results[c]["out"] for c in range(N_CORES)], axis=0)


# revision 37
# speedup vs baseline: 1.1630x; 1.1630x over previous
"""Trainium2 Bass kernel for LongformerForSentenceClassification
(segment-mean pooling over sep-delimited sentences + 3-layer MLP head).

Strategy: data-parallel over the batch dim B=8 across the 8 NeuronCores —
one batch row per core.  The data-dependent segment pooling is expressed as
a dense matmul sent = A @ h, where the (tiny) assignment matrix A
[MAX_SENT, S] is built on the host from input_ids with exactly the
reference semantics (weights, truncation, count normalization).  All heavy
compute runs on-device in fp16 (fp32 PSUM accumulation):

    pooling:  sent[64, 768]   = A[64, 4096] @ h[4096, 768]
    MLP1:     x1[64, 4096]    = gelu(sent @ W1 + b1)
    MLP2:     x2[64, 256]     = gelu(x1 @ W2 + b2)
    MLP3:     logits[64, 2]   = x2 @ W3 + b3

Between layers the activation must be re-laid-out feature-major to serve
as the next matmul's stationary operand (lhsT); those transposes go
through the DMA x-bar (fp16, SBUF->SBUF).  Biases are folded into the
matmul accumulation as K=1 matmuls with a ones-vector lhsT, and skipped
entirely when the host sees an all-zero bias.
"""

import numpy as np

import concourse.bass as bass
import concourse.mybir as mybir
import concourse.tile as tile
from concourse.masks import make_identity
from concourse.vector_clock import ScopedClock
from concourse.bass_utils import run_bass_kernel_spmd

SEP = 2
B, S, H = 8, 4096, 768
MAX_SENT = 64
F1, F2, NCLS = 4096, 256, 2
N_CORES = 8

KS = S // 128          # 32 k-chunks over tokens
KH = H // 128          # 6  k-chunks over hidden dim
KF1 = F1 // 128        # 32 k-chunks over F1
KF2 = F2 // 128        # 2  k-chunks over F2
N1 = F1 // 512         # 8  n-chunks of MLP1 output
HJ = 4                 # h tile granularity: 4 k-chunks per DMA tile
FP16 = mybir.dt.float16
F32 = mybir.dt.float32
GELU = mybir.ActivationFunctionType.Gelu

# exec-time metadata from the most recent kernel() call (filled when
# BASS_TRACE=1); harmless extra attribute for test harnesses.
LAST_META = {}


class SplitDrainTileContext(tile.TileContext):
    """The walrus build in this container only accepts a single sync-wait
    on the kernel-tail Drain instruction; emit the global-clock waits as
    individual wait_ge instructions instead of stacking them on the drain."""

    def _drain_and_barrier(self, tick_clock, wait_clock):
        nc = self.nc
        probe = nc.sync.nop(nofuse=True)
        wait_clock.add_sem_waits(
            probe.ins, ScopedClock({None: tick_clock.global_clock})
        )
        si = probe.ins.sync_info
        waits = list(si.on_wait) if si is not None and si.on_wait else []
        if si is not None and si.on_wait:
            si.on_wait.clear()
        sem_by_num = {s.num: s for s in self.sems.allocated().values()}
        for w in waits:
            assert w.wait_mode == "sem-ge-imm", w
            nc.sync.wait_ge(sem_by_num[w.id], w.wait_value)
        nc.sync.drain()
        nc.all_engine_barrier()
        popped = nc._tile_sem_poison_stack.pop()
        assert popped is self._sem_poison
        nc.clear_and_free_semaphores(list(self.sems.allocated().values()))
        nc.all_engine_barrier()


def _split_multi_waits(nc) -> None:
    """The walrus build here rejects instructions carrying more than one
    sync-wait ("Too many sync wait commands").  Hoist all but the last wait
    of every instruction onto dedicated same-engine NoOps placed directly
    before it — semantically identical (the engine blocks on each wait in
    order before executing the instruction)."""
    for bb in nc.m.functions[0].blocks:
        insts = bb.instructions
        i = 0
        while i < len(insts):
            inst = insts[i]
            si = inst.sync_info
            if si is not None and si.on_wait and len(si.on_wait) > 1:
                extra = list(si.on_wait[:-1])
                keep = si.on_wait[-1]
                si.on_wait.clear()
                si.on_wait.append(keep)
                for j, w in enumerate(extra):
                    nop = mybir.InstNoOp(
                        name=nc.get_next_instruction_name(),
                        sync_info=mybir.SyncInfo(on_wait=[w], on_update=[]),
                        bass_nofuse=True,
                        engine=inst.engine,
                    )
                    nc.register_instruction(nop)
                    insts.insert(i + j, nop)
                i += len(extra)
            i += 1


def _pool_meta(ids: np.ndarray):
    """[B, S] token ids -> (seg_eff [B, S] int32, inv_cnt [B, MAX_SENT] f32)
    matching the reference segment-mean semantics exactly.  seg_eff is the
    clamped segment id, with weight-excluded tokens pointed at the dump
    bucket MAX_SENT; inv_cnt is 1/token-count per sentence (empty -> the
    sums are zero anyway, so the scale value there is irrelevant)."""
    ids = np.asarray(ids)
    sep = ids == SEP
    sep_i = sep.astype(np.int64)
    seg = np.cumsum(sep_i, axis=1) - sep_i          # exclusive cumsum
    n_sep = sep_i.sum(axis=1)                       # [B]
    first_sep = np.argmax(sep, axis=1)              # 0 if no sep at all
    pos = np.arange(ids.shape[1])
    # the first sep belongs to sentence 0; later seps are excluded
    w = np.where(sep, pos[None, :] == first_sep[:, None], True)
    # exclude last token of the trailing (post-last-sep) segment
    w &= ~(
        (pos[None, :] == ids.shape[1] - 1)
        & (seg == n_sep[:, None])
        & (n_sep[:, None] > 0)
    )
    seg_c = np.minimum(seg, MAX_SENT)               # overflow -> dump bucket
    seg_eff = np.where(w, seg_c, MAX_SENT).astype(np.int32)
    cnt = (seg_eff[:, None, :] == np.arange(MAX_SENT)[None, :, None]).sum(axis=2)
    inv_cnt = (1.0 / np.maximum(cnt, 1)).astype(np.float32)
    return seg_eff, inv_cnt


_BUILD_CACHE = {}


def _build(with_b1: bool, with_b2: bool, b3_vals: tuple):
    key = (with_b1, with_b2, b3_vals)
    if key in _BUILD_CACHE:
        return _BUILD_CACHE[key]

    nc = bass.Bass()
    h_d = nc.declare_dram_parameter("h", [128, KS * H], FP16, isOutput=False)
    seg_d = nc.declare_dram_parameter("seg", [128, KS], F32, isOutput=False)
    invc_d = nc.declare_dram_parameter("invc", [MAX_SENT, 1], F32, isOutput=False)
    w1_d = nc.declare_dram_parameter("w1", [128, N1 * KH * 512], FP16, isOutput=False)
    w2_d = nc.declare_dram_parameter("w2", [128, KF1 * F2], FP16, isOutput=False)
    w3_d = nc.declare_dram_parameter(
        "w3", [MAX_SENT, NCLS, F2], FP16, isOutput=False
    )
    b1_d = b2_d = None
    if with_b1:
        b1_d = nc.declare_dram_parameter("b1", [1, F1], FP16, isOutput=False)
    if with_b2:
        b2_d = nc.declare_dram_parameter("b2", [1, F2], FP16, isOutput=False)
    out_d = nc.declare_dram_parameter("out", [MAX_SENT, NCLS], F32, isOutput=True)

    with SplitDrainTileContext(nc) as tc:
        with (
            tc.tile_pool(name="wpool", bufs=1) as wpool,
            tc.tile_pool(name="apool", bufs=1) as apool,
            tc.tile_pool(name="psacc", bufs=1, space="PSUM") as psacc,
            tc.tile_pool(name="ps1", bufs=2, space="PSUM") as ps1pool,
            tc.tile_pool(name="psT", bufs=2, space="PSUM") as psTpool,
        ):
            # [64, 64] identity: rhs operand for PE-mode transposes of
            # [64, 128] activation slices (DMA-xbar transposes would
            # serialize behind the big weight-load DMA stream)
            ident = wpool.tile([MAX_SENT, MAX_SENT], FP16, tag="ident")
            make_identity(nc, ident[:])

            def pe_transpose(dst, src):
                """dst [128, 64] (sbuf) = src [64, 128] (sbuf) transposed."""
                psT = psTpool.tile([128, MAX_SENT], FP16, tag="psT")
                nc.tensor.transpose(psT[:], src, ident[:])
                nc.vector.tensor_copy(out=dst, in_=psT[:])

            # ---- input loads, in consumption order ----
            # build the pooling assignment matrix on-device: at[p, k, m] =
            # (seg_id[token k*128+p] == m), from a 16 KB seg-id tensor
            # (weight-excluded tokens are pre-pointed at the dump id 64 on
            # the host; 1/count normalization is applied at PSUM eviction)
            seg_sb = wpool.tile([128, KS], F32, tag="seg")
            nc.sync.dma_start(out=seg_sb[:], in_=seg_d[:])
            invc_sb = wpool.tile([MAX_SENT, 1], F32, tag="invc")
            nc.sync.dma_start(out=invc_sb[:], in_=invc_d[:])
            iota_sb = wpool.tile([128, MAX_SENT], F32, tag="iota")
            nc.gpsimd.iota(iota_sb[:], pattern=[[1, MAX_SENT]], base=0,
                           channel_multiplier=0,
                           allow_small_or_imprecise_dtypes=True)
            at_sb = wpool.tile([128, KS, MAX_SENT], FP16, tag="at")
            for k in range(KS):
                nc.vector.tensor_scalar(
                    at_sb[:, k, :], iota_sb[:], seg_sb[:, k : k + 1], None,
                    op0=mybir.AluOpType.is_equal,
                )
            h_sb = []
            for j in range(KS // HJ):
                t = wpool.tile([128, HJ, H], FP16, tag=f"h{j}")
                nc.sync.dma_start(
                    out=t[:],
                    in_=h_d[:, j * HJ * H : (j + 1) * HJ * H].rearrange(
                        "p (k h) -> p k h", k=HJ
                    ),
                )
                h_sb.append(t)

            # w3 (tiny, broadcast layout for the DVE/ACT classifier) early
            w3_sb = wpool.tile([MAX_SENT, NCLS, F2], FP16, tag="w3")
            nc.sync.dma_start(out=w3_sb[:], in_=w3_d[:])
            # w1 tile n split into two k-halves so chunk n's first matmuls
            # start half a tile-transfer earlier (shortens the tail chain
            # behind the final w1 bytes)
            w1_sb = []
            for n in range(N1):
                t = wpool.tile([128, KH, 512], FP16, tag=f"w1{n}")
                for half in range(2):
                    k0, k1 = (0, KH // 2) if half == 0 else (KH // 2, KH)
                    nc.sync.dma_start(
                        out=t[:, k0:k1, :],
                        in_=w1_d[
                            :, (n * KH + k0) * 512 : (n * KH + k1) * 512
                        ].rearrange("p (k n) -> p k n", k=k1 - k0),
                    )
                w1_sb.append(t)
            # w2 in quarters: the last bytes of the load stream gate only
            # 8 of MLP2's 32 matmuls
            w2_sb = wpool.tile([128, KF1, F2], FP16, tag="w2")
            w2_pieces = [(0, 8), (8, 16), (16, 24), (24, 28), (28, 32)]
            for k0, k1 in w2_pieces:
                nc.sync.dma_start(
                    out=w2_sb[:, k0:k1, :],
                    in_=w2_d[:, k0 * F2 : k1 * F2].rearrange(
                        "p (k n) -> p k n", k=k1 - k0
                    ),
                )
            ones_sb = b1_sb = b2_sb = None
            if with_b1 or with_b2:
                ones_sb = wpool.tile([1, MAX_SENT], FP16, tag="ones")
                nc.vector.memset(ones_sb[:], 1.0)
            if with_b1:
                b1_sb = wpool.tile([1, F1], FP16, tag="b1")
                nc.sync.dma_start(out=b1_sb[:], in_=b1_d[:])
            if with_b2:
                b2_sb = wpool.tile([1, F2], FP16, tag="b2")
                nc.sync.dma_start(out=b2_sb[:], in_=b2_d[:])

            # ---- pooling: sent = A @ h  -> psum [64, 768] ----
            ps_sent = psacc.tile([MAX_SENT, H], F32, tag="ps_sent")
            for n0, nsz in ((0, 512), (512, 256)):
                for k in range(KS):
                    nc.tensor.matmul(
                        ps_sent[:, n0 : n0 + nsz],
                        lhsT=at_sb[:, k, :],
                        rhs=h_sb[k // HJ][:, k % HJ, n0 : n0 + nsz],
                        start=(k == 0),
                        stop=(k == KS - 1),
                    )
            sent_sb = apool.tile([MAX_SENT, H], FP16, tag="sent")
            nc.scalar.activation(
                sent_sb[:], ps_sent[:], mybir.ActivationFunctionType.Copy,
                bias=0.0, scale=invc_sb[:],
            )
            sentT = apool.tile([128, KH, MAX_SENT], FP16, tag="sentT")
            for k in range(KH):
                pe_transpose(sentT[:, k, :], sent_sb[:, k * 128 : (k + 1) * 128])

            # ---- MLP1: x1 = gelu(sent @ W1 + b1), chunked by 512 cols ----
            x1T = []
            for n in range(N1):
                ps = ps1pool.tile([MAX_SENT, 512], F32, tag="ps_x1")
                for k in range(KH):
                    nc.tensor.matmul(
                        ps[:],
                        lhsT=sentT[:, k, :],
                        rhs=w1_sb[n][:, k, :],
                        start=(k == 0),
                        stop=(k == KH - 1 and not with_b1),
                    )
                if with_b1:
                    nc.tensor.matmul(
                        ps[:],
                        lhsT=ones_sb[:, :],
                        rhs=b1_sb[:, n * 512 : (n + 1) * 512],
                        start=False,
                        stop=True,
                    )
                x1c = apool.tile([MAX_SENT, 512], FP16, tag=f"x1c{n}")
                nc.scalar.activation(x1c[:], ps[:], GELU)
                t = apool.tile([128, HJ, MAX_SENT], FP16, tag=f"x1T{n}")
                for c in range(HJ):
                    pe_transpose(t[:, c, :], x1c[:, c * 128 : (c + 1) * 128])
                x1T.append(t)

            # ---- MLP2: x2 = gelu(x1 @ W2 + b2) ----
            ps2 = psacc.tile([MAX_SENT, F2], F32, tag="ps_x2")
            for k in range(KF1):
                nc.tensor.matmul(
                    ps2[:],
                    lhsT=x1T[k // HJ][:, k % HJ, :],
                    rhs=w2_sb[:, k, :],
                    start=(k == 0),
                    stop=(k == KF1 - 1 and not with_b2),
                )
            if with_b2:
                nc.tensor.matmul(
                    ps2[:], lhsT=ones_sb[:, :], rhs=b2_sb[:, :], start=False, stop=True
                )
            x2_sb = apool.tile([MAX_SENT, F2], FP16, tag="x2")
            nc.scalar.activation(x2_sb[:], ps2[:], GELU)

            # ---- MLP3: logits[t, c] = sum_g x2[t, g] * W3[g, c] + b3[c] ----
            # tiny contraction (256 -> 2): one DVE multiply+reduce per class
            # against a host-broadcast W3, with b3[c] baked as the reduce
            # init — avoids transposing x2, keeps the tail chain short
            out_sb = apool.tile([MAX_SENT, NCLS], F32, tag="outsb")
            for c in range(NCLS):
                tmp = apool.tile([MAX_SENT, F2], FP16, tag=f"mlp3tmp{c}")
                nc.vector.tensor_mul(tmp[:], x2_sb[:], w3_sb[:, c, :])
                nc.vector.tensor_reduce(
                    out_sb[:, c : c + 1],
                    tmp[:],
                    axis=mybir.AxisListType.X,
                    op=mybir.AluOpType.add,
                )
            if any(v != 0.0 for v in b3_vals):
                for c in range(NCLS):
                    nc.vector.tensor_scalar_add(
                        out_sb[:, c : c + 1], out_sb[:, c : c + 1], float(b3_vals[c])
                    )
            nc.sync.dma_start(out=out_d[:], in_=out_sb[:])

    _split_multi_waits(nc)
    _BUILD_CACHE[key] = nc
    return nc


def kernel(hidden, input_ids, W1, b1, W2, b2, W3, b3):
    hidden = np.asarray(hidden, dtype=np.float32)
    W1 = np.asarray(W1, dtype=np.float32)
    W2 = np.asarray(W2, dtype=np.float32)
    W3 = np.asarray(W3, dtype=np.float32)
    b1 = np.asarray(b1, dtype=np.float32)
    b2 = np.asarray(b2, dtype=np.float32)
    b3 = np.asarray(b3, dtype=np.float32)

    seg_eff, inv_cnt = _pool_meta(input_ids)            # [B, S], [B, 64]

    # pack per-core operands [128 partitions, free] so every DMA line is
    # fully contiguous.  token t = k*128 + p; feature f = k*128 + p.
    h16 = hidden.astype(np.float16)
    h_pack = np.ascontiguousarray(
        h16.reshape(B, KS, 128, H).transpose(0, 2, 1, 3)
    ).reshape(B, 128, KS * H)
    seg_pack = np.ascontiguousarray(
        seg_eff.astype(np.float32).reshape(B, KS, 128).transpose(0, 2, 1)
    )                                                   # [B, 128, KS]
    invc_pack = inv_cnt.reshape(B, MAX_SENT, 1)
    w1_pack = np.ascontiguousarray(
        W1.astype(np.float16).reshape(KH, 128, N1, 512).transpose(1, 2, 0, 3)
    ).reshape(128, N1 * KH * 512)
    w2_pack = np.ascontiguousarray(
        W2.astype(np.float16).reshape(KF1, 128, F2).transpose(1, 0, 2)
    ).reshape(128, KF1 * F2)
    # W3 broadcast across the 64 sentence partitions for the DVE classifier
    w3_pack = np.ascontiguousarray(
        np.broadcast_to(
            W3.T.astype(np.float16).reshape(1, NCLS, F2), (MAX_SENT, NCLS, F2)
        )
    )

    with_b1 = bool(np.any(b1))
    with_b2 = bool(np.any(b2))
    nc = _build(with_b1, with_b2, tuple(float(v) for v in b3))

    in_maps = []
    for c in range(N_CORES):
        m = {
            "h": h_pack[c],
            "seg": seg_pack[c],
            "invc": invc_pack[c],
            "w1": w1_pack,
            "w2": w2_pack,
            "w3": w3_pack,
        }
        if with_b1:
            m["b1"] = b1.astype(np.float16).reshape(1, F1)
        if with_b2:
            m["b2"] = b2.astype(np.float16).reshape(1, F2)
        in_maps.append(m)

    res = run_bass_kernel_spmd(nc, in_maps, list(range(N_CORES)))
    LAST_META.clear()
    LAST_META["exec_time_ns"] = res.exec_time_ns
    LAST_META["mean_exec_time_ns"] = res.mean_exec_time_ns
    if res.instructions_and_trace is not None:
        LAST_META["trace"] = res.instructions_and_trace[1]

    return np.stack([res.results[c]["out"] for c in range(N_CORES)], axis=0)


# revision 38
# speedup vs baseline: 1.1780x; 1.0129x over previous
"""Trainium2 Bass kernel for LongformerForSentenceClassification
(segment-mean pooling over sep-delimited sentences + 3-layer MLP head).

Strategy: data-parallel over the batch dim B=8 across the 8 NeuronCores —
one batch row per core.  The data-dependent segment pooling is expressed as
a dense matmul sent = A @ h, where the (tiny) assignment matrix A
[MAX_SENT, S] is built on the host from input_ids with exactly the
reference semantics (weights, truncation, count normalization).  All heavy
compute runs on-device in fp16 (fp32 PSUM accumulation):

    pooling:  sent[64, 768]   = A[64, 4096] @ h[4096, 768]
    MLP1:     x1[64, 4096]    = gelu(sent @ W1 + b1)
    MLP2:     x2[64, 256]     = gelu(x1 @ W2 + b2)
    MLP3:     logits[64, 2]   = x2 @ W3 + b3

Between layers the activation must be re-laid-out feature-major to serve
as the next matmul's stationary operand (lhsT); those transposes go
through the DMA x-bar (fp16, SBUF->SBUF).  Biases are folded into the
matmul accumulation as K=1 matmuls with a ones-vector lhsT, and skipped
entirely when the host sees an all-zero bias.
"""

import numpy as np

import concourse.bass as bass
import concourse.mybir as mybir
import concourse.tile as tile
from concourse.masks import make_identity
from concourse.vector_clock import ScopedClock
from concourse.bass_utils import run_bass_kernel_spmd

SEP = 2
B, S, H = 8, 4096, 768
MAX_SENT = 64
F1, F2, NCLS = 4096, 256, 2
N_CORES = 8

KS = S // 128          # 32 k-chunks over tokens
KH = H // 128          # 6  k-chunks over hidden dim
KF1 = F1 // 128        # 32 k-chunks over F1
KF2 = F2 // 128        # 2  k-chunks over F2
N1 = F1 // 512         # 8  n-chunks of MLP1 output
HJ = 4                 # h tile granularity: 4 k-chunks per DMA tile
FP16 = mybir.dt.float16
F32 = mybir.dt.float32
GELU = mybir.ActivationFunctionType.Gelu

# exec-time metadata from the most recent kernel() call (filled when
# BASS_TRACE=1); harmless extra attribute for test harnesses.
LAST_META = {}


class SplitDrainTileContext(tile.TileContext):
    """The walrus build in this container only accepts a single sync-wait
    on the kernel-tail Drain instruction; emit the global-clock waits as
    individual wait_ge instructions instead of stacking them on the drain."""

    def _drain_and_barrier(self, tick_clock, wait_clock):
        nc = self.nc
        probe = nc.sync.nop(nofuse=True)
        wait_clock.add_sem_waits(
            probe.ins, ScopedClock({None: tick_clock.global_clock})
        )
        si = probe.ins.sync_info
        waits = list(si.on_wait) if si is not None and si.on_wait else []
        if si is not None and si.on_wait:
            si.on_wait.clear()
        sem_by_num = {s.num: s for s in self.sems.allocated().values()}
        for w in waits:
            assert w.wait_mode == "sem-ge-imm", w
            nc.sync.wait_ge(sem_by_num[w.id], w.wait_value)
        nc.sync.drain()
        nc.all_engine_barrier()
        popped = nc._tile_sem_poison_stack.pop()
        assert popped is self._sem_poison
        nc.clear_and_free_semaphores(list(self.sems.allocated().values()))
        nc.all_engine_barrier()


def _split_multi_waits(nc) -> None:
    """The walrus build here rejects instructions carrying more than one
    sync-wait ("Too many sync wait commands").  Hoist all but the last wait
    of every instruction onto dedicated same-engine NoOps placed directly
    before it — semantically identical (the engine blocks on each wait in
    order before executing the instruction)."""
    for bb in nc.m.functions[0].blocks:
        insts = bb.instructions
        i = 0
        while i < len(insts):
            inst = insts[i]
            si = inst.sync_info
            if si is not None and si.on_wait and len(si.on_wait) > 1:
                extra = list(si.on_wait[:-1])
                keep = si.on_wait[-1]
                si.on_wait.clear()
                si.on_wait.append(keep)
                for j, w in enumerate(extra):
                    nop = mybir.InstNoOp(
                        name=nc.get_next_instruction_name(),
                        sync_info=mybir.SyncInfo(on_wait=[w], on_update=[]),
                        bass_nofuse=True,
                        engine=inst.engine,
                    )
                    nc.register_instruction(nop)
                    insts.insert(i + j, nop)
                i += len(extra)
            i += 1


def _pool_meta(ids: np.ndarray):
    """[B, S] token ids -> (seg_eff [B, S] int32, inv_cnt [B, MAX_SENT] f32)
    matching the reference segment-mean semantics exactly.  seg_eff is the
    clamped segment id, with weight-excluded tokens pointed at the dump
    bucket MAX_SENT; inv_cnt is 1/token-count per sentence (empty -> the
    sums are zero anyway, so the scale value there is irrelevant)."""
    ids = np.asarray(ids)
    sep = ids == SEP
    sep_i = sep.astype(np.int64)
    seg = np.cumsum(sep_i, axis=1) - sep_i          # exclusive cumsum
    n_sep = sep_i.sum(axis=1)                       # [B]
    first_sep = np.argmax(sep, axis=1)              # 0 if no sep at all
    pos = np.arange(ids.shape[1])
    # the first sep belongs to sentence 0; later seps are excluded
    w = np.where(sep, pos[None, :] == first_sep[:, None], True)
    # exclude last token of the trailing (post-last-sep) segment
    w &= ~(
        (pos[None, :] == ids.shape[1] - 1)
        & (seg == n_sep[:, None])
        & (n_sep[:, None] > 0)
    )
    seg_c = np.minimum(seg, MAX_SENT)               # overflow -> dump bucket
    seg_eff = np.where(w, seg_c, MAX_SENT).astype(np.int32)
    cnt = (seg_eff[:, None, :] == np.arange(MAX_SENT)[None, :, None]).sum(axis=2)
    inv_cnt = (1.0 / np.maximum(cnt, 1)).astype(np.float32)
    return seg_eff, inv_cnt


_BUILD_CACHE = {}


def _build(with_b1: bool, with_b2: bool, b3_vals: tuple):
    key = (with_b1, with_b2, b3_vals)
    if key in _BUILD_CACHE:
        return _BUILD_CACHE[key]

    nc = bass.Bass()
    h_d = nc.declare_dram_parameter("h", [128, KS * H], FP16, isOutput=False)
    seg_d = nc.declare_dram_parameter("seg", [128, KS + 1], F32, isOutput=False)
    w1_d = nc.declare_dram_parameter("w1", [128, N1 * KH * 512], FP16, isOutput=False)
    w2_d = nc.declare_dram_parameter("w2", [128, KF1 * F2], FP16, isOutput=False)
    w3_d = nc.declare_dram_parameter(
        "w3", [MAX_SENT, NCLS, F2], FP16, isOutput=False
    )
    b1_d = b2_d = None
    if with_b1:
        b1_d = nc.declare_dram_parameter("b1", [1, F1], FP16, isOutput=False)
    if with_b2:
        b2_d = nc.declare_dram_parameter("b2", [1, F2], FP16, isOutput=False)
    out_d = nc.declare_dram_parameter("out", [MAX_SENT, NCLS], F32, isOutput=True)

    with SplitDrainTileContext(nc) as tc:
        with (
            tc.tile_pool(name="wpool", bufs=1) as wpool,
            tc.tile_pool(name="apool", bufs=1) as apool,
            tc.tile_pool(name="psacc", bufs=1, space="PSUM") as psacc,
            tc.tile_pool(name="ps1", bufs=2, space="PSUM") as ps1pool,
            tc.tile_pool(name="psT", bufs=2, space="PSUM") as psTpool,
        ):
            # [64, 64] identity: rhs operand for PE-mode transposes of
            # [64, 128] activation slices (DMA-xbar transposes would
            # serialize behind the big weight-load DMA stream)
            ident = wpool.tile([MAX_SENT, MAX_SENT], FP16, tag="ident")
            make_identity(nc, ident[:])

            def pe_transpose(dst, src):
                """dst [128, 64] (sbuf) = src [64, 128] (sbuf) transposed."""
                psT = psTpool.tile([128, MAX_SENT], FP16, tag="psT")
                nc.tensor.transpose(psT[:], src, ident[:])
                nc.vector.tensor_copy(out=dst, in_=psT[:])

            # ---- input loads, in consumption order ----
            # build the pooling assignment matrix on-device: at[p, k, m] =
            # (seg_id[token k*128+p] == m), from a 16 KB seg-id tensor
            # (weight-excluded tokens are pre-pointed at the dump id 64 on
            # the host; 1/count normalization is applied at PSUM eviction)
            # seg ids cols 0..KS-1; col KS carries 1/count on partitions
            # 0..63 (merged into one DMA: a tiny transfer ahead of the h
            # stream exposes its full descriptor latency as a bubble)
            seg_sb = wpool.tile([128, KS + 1], F32, tag="seg")
            nc.sync.dma_start(out=seg_sb[:], in_=seg_d[:])
            invc_sb = seg_sb
            iota_sb = wpool.tile([128, MAX_SENT], F32, tag="iota")
            nc.gpsimd.iota(iota_sb[:], pattern=[[1, MAX_SENT]], base=0,
                           channel_multiplier=0,
                           allow_small_or_imprecise_dtypes=True)
            at_sb = wpool.tile([128, KS, MAX_SENT], FP16, tag="at")
            for k in range(KS):
                nc.vector.tensor_scalar(
                    at_sb[:, k, :], iota_sb[:], seg_sb[:, k : k + 1], None,
                    op0=mybir.AluOpType.is_equal,
                )
            h_sb = []
            for j in range(KS // HJ):
                t = wpool.tile([128, HJ, H], FP16, tag=f"h{j}")
                nc.sync.dma_start(
                    out=t[:],
                    in_=h_d[:, j * HJ * H : (j + 1) * HJ * H].rearrange(
                        "p (k h) -> p k h", k=HJ
                    ),
                )
                h_sb.append(t)

            # w3 (tiny, broadcast layout for the DVE/ACT classifier) early
            w3_sb = wpool.tile([MAX_SENT, NCLS, F2], FP16, tag="w3")
            nc.sync.dma_start(out=w3_sb[:], in_=w3_d[:])
            # w1 tile n split into two k-halves so chunk n's first matmuls
            # start half a tile-transfer earlier (shortens the tail chain
            # behind the final w1 bytes)
            w1_sb = []
            for n in range(N1):
                t = wpool.tile([128, KH, 512], FP16, tag=f"w1{n}")
                for half in range(2):
                    k0, k1 = (0, KH // 2) if half == 0 else (KH // 2, KH)
                    nc.sync.dma_start(
                        out=t[:, k0:k1, :],
                        in_=w1_d[
                            :, (n * KH + k0) * 512 : (n * KH + k1) * 512
                        ].rearrange("p (k n) -> p k n", k=k1 - k0),
                    )
                w1_sb.append(t)
            # w2 in quarters: the last bytes of the load stream gate only
            # 8 of MLP2's 32 matmuls
            w2_sb = wpool.tile([128, KF1, F2], FP16, tag="w2")
            w2_pieces = [(0, 8), (8, 16), (16, 24), (24, 28), (28, 30), (30, 32)]
            for k0, k1 in w2_pieces:
                nc.sync.dma_start(
                    out=w2_sb[:, k0:k1, :],
                    in_=w2_d[:, k0 * F2 : k1 * F2].rearrange(
                        "p (k n) -> p k n", k=k1 - k0
                    ),
                )
            ones_sb = b1_sb = b2_sb = None
            if with_b1 or with_b2:
                ones_sb = wpool.tile([1, MAX_SENT], FP16, tag="ones")
                nc.vector.memset(ones_sb[:], 1.0)
            if with_b1:
                b1_sb = wpool.tile([1, F1], FP16, tag="b1")
                nc.sync.dma_start(out=b1_sb[:], in_=b1_d[:])
            if with_b2:
                b2_sb = wpool.tile([1, F2], FP16, tag="b2")
                nc.sync.dma_start(out=b2_sb[:], in_=b2_d[:])

            # ---- pooling: sent = A @ h  -> psum [64, 768] ----
            ps_sent = psacc.tile([MAX_SENT, H], F32, tag="ps_sent")
            for n0, nsz in ((0, 512), (512, 256)):
                for k in range(KS):
                    nc.tensor.matmul(
                        ps_sent[:, n0 : n0 + nsz],
                        lhsT=at_sb[:, k, :],
                        rhs=h_sb[k // HJ][:, k % HJ, n0 : n0 + nsz],
                        start=(k == 0),
                        stop=(k == KS - 1),
                    )
            sent_sb = apool.tile([MAX_SENT, H], FP16, tag="sent")
            nc.scalar.activation(
                sent_sb[:], ps_sent[:], mybir.ActivationFunctionType.Copy,
                bias=0.0, scale=invc_sb[0:MAX_SENT, KS : KS + 1],
            )
            sentT = apool.tile([128, KH, MAX_SENT], FP16, tag="sentT")
            for k in range(KH):
                pe_transpose(sentT[:, k, :], sent_sb[:, k * 128 : (k + 1) * 128])

            # ---- MLP1: x1 = gelu(sent @ W1 + b1), chunked by 512 cols ----
            x1T = []
            for n in range(N1):
                ps = ps1pool.tile([MAX_SENT, 512], F32, tag="ps_x1")
                for k in range(KH):
                    nc.tensor.matmul(
                        ps[:],
                        lhsT=sentT[:, k, :],
                        rhs=w1_sb[n][:, k, :],
                        start=(k == 0),
                        stop=(k == KH - 1 and not with_b1),
                    )
                if with_b1:
                    nc.tensor.matmul(
                        ps[:],
                        lhsT=ones_sb[:, :],
                        rhs=b1_sb[:, n * 512 : (n + 1) * 512],
                        start=False,
                        stop=True,
                    )
                x1c = apool.tile([MAX_SENT, 512], FP16, tag=f"x1c{n}")
                nc.scalar.activation(x1c[:], ps[:], GELU)
                t = apool.tile([128, HJ, MAX_SENT], FP16, tag=f"x1T{n}")
                for c in range(HJ):
                    pe_transpose(t[:, c, :], x1c[:, c * 128 : (c + 1) * 128])
                x1T.append(t)

            # ---- MLP2: x2 = gelu(x1 @ W2 + b2) ----
            ps2 = psacc.tile([MAX_SENT, F2], F32, tag="ps_x2")
            for k in range(KF1):
                nc.tensor.matmul(
                    ps2[:],
                    lhsT=x1T[k // HJ][:, k % HJ, :],
                    rhs=w2_sb[:, k, :],
                    start=(k == 0),
                    stop=(k == KF1 - 1 and not with_b2),
                )
            if with_b2:
                nc.tensor.matmul(
                    ps2[:], lhsT=ones_sb[:, :], rhs=b2_sb[:, :], start=False, stop=True
                )
            x2_sb = apool.tile([MAX_SENT, F2], FP16, tag="x2")
            nc.scalar.activation(x2_sb[:], ps2[:], GELU)

            # ---- MLP3: logits[t, c] = sum_g x2[t, g] * W3[g, c] + b3[c] ----
            # tiny contraction (256 -> 2): one DVE multiply+reduce per class
            # against a host-broadcast W3, with b3[c] baked as the reduce
            # init — avoids transposing x2, keeps the tail chain short
            out_sb = apool.tile([MAX_SENT, NCLS], F32, tag="outsb")
            for c in range(NCLS):
                tmp = apool.tile([MAX_SENT, F2], FP16, tag=f"mlp3tmp{c}")
                nc.vector.tensor_mul(tmp[:], x2_sb[:], w3_sb[:, c, :])
                nc.vector.tensor_reduce(
                    out_sb[:, c : c + 1],
                    tmp[:],
                    axis=mybir.AxisListType.X,
                    op=mybir.AluOpType.add,
                )
            if any(v != 0.0 for v in b3_vals):
                for c in range(NCLS):
                    nc.vector.tensor_scalar_add(
                        out_sb[:, c : c + 1], out_sb[:, c : c + 1], float(b3_vals[c])
                    )
            nc.sync.dma_start(out=out_d[:], in_=out_sb[:])

    _split_multi_waits(nc)
    _BUILD_CACHE[key] = nc
    return nc


def kernel(hidden, input_ids, W1, b1, W2, b2, W3, b3):
    hidden = np.asarray(hidden, dtype=np.float32)
    W1 = np.asarray(W1, dtype=np.float32)
    W2 = np.asarray(W2, dtype=np.float32)
    W3 = np.asarray(W3, dtype=np.float32)
    b1 = np.asarray(b1, dtype=np.float32)
    b2 = np.asarray(b2, dtype=np.float32)
    b3 = np.asarray(b3, dtype=np.float32)

    seg_eff, inv_cnt = _pool_meta(input_ids)            # [B, S], [B, 64]

    # pack per-core operands [128 partitions, free] so every DMA line is
    # fully contiguous.  token t = k*128 + p; feature f = k*128 + p.
    h16 = hidden.astype(np.float16)
    h_pack = np.ascontiguousarray(
        h16.reshape(B, KS, 128, H).transpose(0, 2, 1, 3)
    ).reshape(B, 128, KS * H)
    seg_pack = np.zeros((B, 128, KS + 1), np.float32)
    seg_pack[:, :, :KS] = seg_eff.astype(np.float32).reshape(B, KS, 128).transpose(0, 2, 1)
    seg_pack[:, :MAX_SENT, KS] = inv_cnt
    w1_pack = np.ascontiguousarray(
        W1.astype(np.float16).reshape(KH, 128, N1, 512).transpose(1, 2, 0, 3)
    ).reshape(128, N1 * KH * 512)
    w2_pack = np.ascontiguousarray(
        W2.astype(np.float16).reshape(KF1, 128, F2).transpose(1, 0, 2)
    ).reshape(128, KF1 * F2)
    # W3 broadcast across the 64 sentence partitions for the DVE classifier
    w3_pack = np.ascontiguousarray(
        np.broadcast_to(
            W3.T.astype(np.float16).reshape(1, NCLS, F2), (MAX_SENT, NCLS, F2)
        )
    )

    with_b1 = bool(np.any(b1))
    with_b2 = bool(np.any(b2))
    nc = _build(with_b1, with_b2, tuple(float(v) for v in b3))

    in_maps = []
    for c in range(N_CORES):
        m = {
            "h": h_pack[c],
            "seg": seg_pack[c],
            "w1": w1_pack,
            "w2": w2_pack,
            "w3": w3_pack,
        }
        if with_b1:
            m["b1"] = b1.astype(np.float16).reshape(1, F1)
        if with_b2:
            m["b2"] = b2.astype(np.float16).reshape(1, F2)
        in_maps.append(m)

    res = run_bass_kernel_spmd(nc, in_maps, list(range(N_CORES)))
    LAST_META.clear()
    LAST_META["exec_time_ns"] = res.exec_time_ns
    LAST_META["mean_exec_time_ns"] = res.mean_exec_time_ns
    if res.instructions_and_trace is not None:
        LAST_META["trace"] = res.instructions_and_trace[1]

    return np.stack([res.results[c]["out"] for c in range(N_CORES)], axis=0)


# revision 39
# speedup vs baseline: 1.1911x; 1.0111x over previous
"""Trainium2 Bass kernel for LongformerForSentenceClassification
(segment-mean pooling over sep-delimited sentences + 3-layer MLP head).

Strategy: data-parallel over the batch dim B=8 across the 8 NeuronCores —
one batch row per core.  The data-dependent segment pooling is expressed as
a dense matmul sent = A @ h, where the (tiny) assignment matrix A
[MAX_SENT, S] is built on the host from input_ids with exactly the
reference semantics (weights, truncation, count normalization).  All heavy
compute runs on-device in fp16 (fp32 PSUM accumulation):

    pooling:  sent[64, 768]   = A[64, 4096] @ h[4096, 768]
    MLP1:     x1[64, 4096]    = gelu(sent @ W1 + b1)
    MLP2:     x2[64, 256]     = gelu(x1 @ W2 + b2)
    MLP3:     logits[64, 2]   = x2 @ W3 + b3

Between layers the activation must be re-laid-out feature-major to serve
as the next matmul's stationary operand (lhsT); those transposes go
through the DMA x-bar (fp16, SBUF->SBUF).  Biases are folded into the
matmul accumulation as K=1 matmuls with a ones-vector lhsT, and skipped
entirely when the host sees an all-zero bias.
"""

import numpy as np

import concourse.bass as bass
import concourse.mybir as mybir
import concourse.tile as tile
from concourse.masks import make_identity
from concourse.vector_clock import ScopedClock
from concourse.bass_utils import run_bass_kernel_spmd

SEP = 2
B, S, H = 8, 4096, 768
MAX_SENT = 64
F1, F2, NCLS = 4096, 256, 2
N_CORES = 8

KS = S // 128          # 32 k-chunks over tokens
KH = H // 128          # 6  k-chunks over hidden dim
KF1 = F1 // 128        # 32 k-chunks over F1
KF2 = F2 // 128        # 2  k-chunks over F2
N1 = F1 // 512         # 8  n-chunks of MLP1 output
HJ = 4                 # h tile granularity: 4 k-chunks per DMA tile
FP16 = mybir.dt.float16
F32 = mybir.dt.float32
GELU = mybir.ActivationFunctionType.Gelu

# exec-time metadata from the most recent kernel() call (filled when
# BASS_TRACE=1); harmless extra attribute for test harnesses.
LAST_META = {}


class SplitDrainTileContext(tile.TileContext):
    """The walrus build in this container only accepts a single sync-wait
    on the kernel-tail Drain instruction; emit the global-clock waits as
    individual wait_ge instructions instead of stacking them on the drain."""

    def _drain_and_barrier(self, tick_clock, wait_clock):
        nc = self.nc
        probe = nc.sync.nop(nofuse=True)
        wait_clock.add_sem_waits(
            probe.ins, ScopedClock({None: tick_clock.global_clock})
        )
        si = probe.ins.sync_info
        waits = list(si.on_wait) if si is not None and si.on_wait else []
        if si is not None and si.on_wait:
            si.on_wait.clear()
        sem_by_num = {s.num: s for s in self.sems.allocated().values()}
        for w in waits:
            assert w.wait_mode == "sem-ge-imm", w
            nc.sync.wait_ge(sem_by_num[w.id], w.wait_value)
        nc.sync.drain()
        nc.all_engine_barrier()
        popped = nc._tile_sem_poison_stack.pop()
        assert popped is self._sem_poison
        nc.clear_and_free_semaphores(list(self.sems.allocated().values()))
        nc.all_engine_barrier()


def _split_multi_waits(nc) -> None:
    """The walrus build here rejects instructions carrying more than one
    sync-wait ("Too many sync wait commands").  Hoist all but the last wait
    of every instruction onto dedicated same-engine NoOps placed directly
    before it — semantically identical (the engine blocks on each wait in
    order before executing the instruction)."""
    for bb in nc.m.functions[0].blocks:
        insts = bb.instructions
        i = 0
        while i < len(insts):
            inst = insts[i]
            si = inst.sync_info
            if si is not None and si.on_wait and len(si.on_wait) > 1:
                extra = list(si.on_wait[:-1])
                keep = si.on_wait[-1]
                si.on_wait.clear()
                si.on_wait.append(keep)
                for j, w in enumerate(extra):
                    nop = mybir.InstNoOp(
                        name=nc.get_next_instruction_name(),
                        sync_info=mybir.SyncInfo(on_wait=[w], on_update=[]),
                        bass_nofuse=True,
                        engine=inst.engine,
                    )
                    nc.register_instruction(nop)
                    insts.insert(i + j, nop)
                i += len(extra)
            i += 1


def _pool_meta(ids: np.ndarray):
    """[B, S] token ids -> (seg_eff [B, S] int32, inv_cnt [B, MAX_SENT] f32)
    matching the reference segment-mean semantics exactly.  seg_eff is the
    clamped segment id, with weight-excluded tokens pointed at the dump
    bucket MAX_SENT; inv_cnt is 1/token-count per sentence (empty -> the
    sums are zero anyway, so the scale value there is irrelevant)."""
    ids = np.asarray(ids)
    sep = ids == SEP
    sep_i = sep.astype(np.int64)
    seg = np.cumsum(sep_i, axis=1) - sep_i          # exclusive cumsum
    n_sep = sep_i.sum(axis=1)                       # [B]
    first_sep = np.argmax(sep, axis=1)              # 0 if no sep at all
    pos = np.arange(ids.shape[1])
    # the first sep belongs to sentence 0; later seps are excluded
    w = np.where(sep, pos[None, :] == first_sep[:, None], True)
    # exclude last token of the trailing (post-last-sep) segment
    w &= ~(
        (pos[None, :] == ids.shape[1] - 1)
        & (seg == n_sep[:, None])
        & (n_sep[:, None] > 0)
    )
    seg_c = np.minimum(seg, MAX_SENT)               # overflow -> dump bucket
    seg_eff = np.where(w, seg_c, MAX_SENT).astype(np.int32)
    cnt = (seg_eff[:, None, :] == np.arange(MAX_SENT)[None, :, None]).sum(axis=2)
    inv_cnt = (1.0 / np.maximum(cnt, 1)).astype(np.float32)
    return seg_eff, inv_cnt


_BUILD_CACHE = {}


def _build(with_b1: bool, with_b2: bool, b3_vals: tuple):
    key = (with_b1, with_b2, b3_vals)
    if key in _BUILD_CACHE:
        return _BUILD_CACHE[key]

    nc = bass.Bass()
    h_d = nc.declare_dram_parameter("h", [128, KS * H], FP16, isOutput=False)
    seg_d = nc.declare_dram_parameter("seg", [128, KS + 1], F32, isOutput=False)
    w1_d = nc.declare_dram_parameter("w1", [128, N1 * KH * 512], FP16, isOutput=False)
    w2_d = nc.declare_dram_parameter("w2", [128, KF1 * F2], FP16, isOutput=False)
    w3_d = nc.declare_dram_parameter(
        "w3", [MAX_SENT, NCLS, F2], FP16, isOutput=False
    )
    b1_d = b2_d = None
    if with_b1:
        b1_d = nc.declare_dram_parameter("b1", [1, F1], FP16, isOutput=False)
    if with_b2:
        b2_d = nc.declare_dram_parameter("b2", [1, F2], FP16, isOutput=False)
    out_d = nc.declare_dram_parameter("out", [MAX_SENT, NCLS], F32, isOutput=True)

    with SplitDrainTileContext(nc) as tc:
        with (
            tc.tile_pool(name="wpool", bufs=1) as wpool,
            tc.tile_pool(name="apool", bufs=1) as apool,
            tc.tile_pool(name="psacc", bufs=1, space="PSUM") as psacc,
            tc.tile_pool(name="ps1", bufs=2, space="PSUM") as ps1pool,
            tc.tile_pool(name="psT", bufs=2, space="PSUM") as psTpool,
        ):
            # [64, 64] identity: rhs operand for PE-mode transposes of
            # [64, 128] activation slices (DMA-xbar transposes would
            # serialize behind the big weight-load DMA stream)
            ident = wpool.tile([MAX_SENT, MAX_SENT], FP16, tag="ident")
            make_identity(nc, ident[:])

            def pe_transpose(dst, src):
                """dst [128, 64] (sbuf) = src [64, 128] (sbuf) transposed."""
                psT = psTpool.tile([128, MAX_SENT], FP16, tag="psT")
                nc.tensor.transpose(psT[:], src, ident[:])
                nc.vector.tensor_copy(out=dst, in_=psT[:])

            # ---- input loads, in consumption order ----
            # build the pooling assignment matrix on-device: at[p, k, m] =
            # (seg_id[token k*128+p] == m), from a 16 KB seg-id tensor
            # (weight-excluded tokens are pre-pointed at the dump id 64 on
            # the host; 1/count normalization is applied at PSUM eviction)
            # first h tile goes ahead of everything: its 2.2 us transfer
            # hides the tiny seg DMA's descriptor latency
            h_sb = []
            t0 = wpool.tile([128, HJ, H], FP16, tag="h0")
            nc.sync.dma_start(
                out=t0[:],
                in_=h_d[:, : HJ * H].rearrange("p (k h) -> p k h", k=HJ),
            )
            h_sb.append(t0)
            # seg ids cols 0..KS-1; col KS carries 1/count on partitions
            # 0..63 (merged into one DMA)
            seg_sb = wpool.tile([128, KS + 1], F32, tag="seg")
            nc.sync.dma_start(out=seg_sb[:], in_=seg_d[:])
            invc_sb = seg_sb
            iota_sb = wpool.tile([128, MAX_SENT], F32, tag="iota")
            nc.gpsimd.iota(iota_sb[:], pattern=[[1, MAX_SENT]], base=0,
                           channel_multiplier=0,
                           allow_small_or_imprecise_dtypes=True)
            at_sb = wpool.tile([128, KS, MAX_SENT], FP16, tag="at")
            for k in range(KS):
                nc.vector.tensor_scalar(
                    at_sb[:, k, :], iota_sb[:], seg_sb[:, k : k + 1], None,
                    op0=mybir.AluOpType.is_equal,
                )
            for j in range(1, KS // HJ):
                t = wpool.tile([128, HJ, H], FP16, tag=f"h{j}")
                nc.sync.dma_start(
                    out=t[:],
                    in_=h_d[:, j * HJ * H : (j + 1) * HJ * H].rearrange(
                        "p (k h) -> p k h", k=HJ
                    ),
                )
                h_sb.append(t)

            # w3 (tiny, broadcast layout for the DVE/ACT classifier) early
            w3_sb = wpool.tile([MAX_SENT, NCLS, F2], FP16, tag="w3")
            nc.sync.dma_start(out=w3_sb[:], in_=w3_d[:])
            # w1 tile n split into two k-halves so chunk n's first matmuls
            # start half a tile-transfer earlier (shortens the tail chain
            # behind the final w1 bytes)
            w1_sb = []
            for n in range(N1):
                t = wpool.tile([128, KH, 512], FP16, tag=f"w1{n}")
                for half in range(2):
                    k0, k1 = (0, KH // 2) if half == 0 else (KH // 2, KH)
                    nc.sync.dma_start(
                        out=t[:, k0:k1, :],
                        in_=w1_d[
                            :, (n * KH + k0) * 512 : (n * KH + k1) * 512
                        ].rearrange("p (k n) -> p k n", k=k1 - k0),
                    )
                w1_sb.append(t)
            # w2 in quarters: the last bytes of the load stream gate only
            # 8 of MLP2's 32 matmuls
            w2_sb = wpool.tile([128, KF1, F2], FP16, tag="w2")
            w2_pieces = [(0, 8), (8, 16), (16, 24), (24, 28), (28, 30), (30, 32)]
            for k0, k1 in w2_pieces:
                nc.sync.dma_start(
                    out=w2_sb[:, k0:k1, :],
                    in_=w2_d[:, k0 * F2 : k1 * F2].rearrange(
                        "p (k n) -> p k n", k=k1 - k0
                    ),
                )
            ones_sb = b1_sb = b2_sb = None
            if with_b1 or with_b2:
                ones_sb = wpool.tile([1, MAX_SENT], FP16, tag="ones")
                nc.vector.memset(ones_sb[:], 1.0)
            if with_b1:
                b1_sb = wpool.tile([1, F1], FP16, tag="b1")
                nc.sync.dma_start(out=b1_sb[:], in_=b1_d[:])
            if with_b2:
                b2_sb = wpool.tile([1, F2], FP16, tag="b2")
                nc.sync.dma_start(out=b2_sb[:], in_=b2_d[:])

            # ---- pooling: sent = A @ h  -> psum [64, 768] ----
            ps_sent = psacc.tile([MAX_SENT, H], F32, tag="ps_sent")
            for n0, nsz in ((0, 512), (512, 256)):
                for k in range(KS):
                    nc.tensor.matmul(
                        ps_sent[:, n0 : n0 + nsz],
                        lhsT=at_sb[:, k, :],
                        rhs=h_sb[k // HJ][:, k % HJ, n0 : n0 + nsz],
                        start=(k == 0),
                        stop=(k == KS - 1),
                    )
            sent_sb = apool.tile([MAX_SENT, H], FP16, tag="sent")
            nc.scalar.activation(
                sent_sb[:], ps_sent[:], mybir.ActivationFunctionType.Copy,
                bias=0.0, scale=invc_sb[0:MAX_SENT, KS : KS + 1],
            )
            sentT = apool.tile([128, KH, MAX_SENT], FP16, tag="sentT")
            for k in range(KH):
                pe_transpose(sentT[:, k, :], sent_sb[:, k * 128 : (k + 1) * 128])

            # ---- MLP1: x1 = gelu(sent @ W1 + b1), chunked by 512 cols ----
            x1T = []
            for n in range(N1):
                ps = ps1pool.tile([MAX_SENT, 512], F32, tag="ps_x1")
                for k in range(KH):
                    nc.tensor.matmul(
                        ps[:],
                        lhsT=sentT[:, k, :],
                        rhs=w1_sb[n][:, k, :],
                        start=(k == 0),
                        stop=(k == KH - 1 and not with_b1),
                    )
                if with_b1:
                    nc.tensor.matmul(
                        ps[:],
                        lhsT=ones_sb[:, :],
                        rhs=b1_sb[:, n * 512 : (n + 1) * 512],
                        start=False,
                        stop=True,
                    )
                x1c = apool.tile([MAX_SENT, 512], FP16, tag=f"x1c{n}")
                nc.scalar.activation(x1c[:], ps[:], GELU)
                t = apool.tile([128, HJ, MAX_SENT], FP16, tag=f"x1T{n}")
                for c in range(HJ):
                    pe_transpose(t[:, c, :], x1c[:, c * 128 : (c + 1) * 128])
                x1T.append(t)

            # ---- MLP2: x2 = gelu(x1 @ W2 + b2) ----
            ps2 = psacc.tile([MAX_SENT, F2], F32, tag="ps_x2")
            for k in range(KF1):
                nc.tensor.matmul(
                    ps2[:],
                    lhsT=x1T[k // HJ][:, k % HJ, :],
                    rhs=w2_sb[:, k, :],
                    start=(k == 0),
                    stop=(k == KF1 - 1 and not with_b2),
                )
            if with_b2:
                nc.tensor.matmul(
                    ps2[:], lhsT=ones_sb[:, :], rhs=b2_sb[:, :], start=False, stop=True
                )
            x2_sb = apool.tile([MAX_SENT, F2], FP16, tag="x2")
            nc.scalar.activation(x2_sb[:], ps2[:], GELU)

            # ---- MLP3: logits[t, c] = sum_g x2[t, g] * W3[g, c] + b3[c] ----
            # tiny contraction (256 -> 2): one DVE multiply+reduce per class
            # against a host-broadcast W3, with b3[c] baked as the reduce
            # init — avoids transposing x2, keeps the tail chain short
            out_sb = apool.tile([MAX_SENT, NCLS], F32, tag="outsb")
            for c in range(NCLS):
                tmp = apool.tile([MAX_SENT, F2], FP16, tag=f"mlp3tmp{c}")
                nc.vector.tensor_mul(tmp[:], x2_sb[:], w3_sb[:, c, :])
                nc.vector.tensor_reduce(
                    out_sb[:, c : c + 1],
                    tmp[:],
                    axis=mybir.AxisListType.X,
                    op=mybir.AluOpType.add,
                )
            if any(v != 0.0 for v in b3_vals):
                for c in range(NCLS):
                    nc.vector.tensor_scalar_add(
                        out_sb[:, c : c + 1], out_sb[:, c : c + 1], float(b3_vals[c])
                    )
            nc.sync.dma_start(out=out_d[:], in_=out_sb[:])

    _split_multi_waits(nc)
    _BUILD_CACHE[key] = nc
    return nc


def kernel(hidden, input_ids, W1, b1, W2, b2, W3, b3):
    hidden = np.asarray(hidden, dtype=np.float32)
    W1 = np.asarray(W1, dtype=np.float32)
    W2 = np.asarray(W2, dtype=np.float32)
    W3 = np.asarray(W3, dtype=np.float32)
    b1 = np.asarray(b1, dtype=np.float32)
    b2 = np.asarray(b2, dtype=np.float32)
    b3 = np.asarray(b3, dtype=np.float32)

    seg_eff, inv_cnt = _pool_meta(input_ids)            # [B, S], [B, 64]

    # pack per-core operands [128 partitions, free] so every DMA line is
    # fully contiguous.  token t = k*128 + p; feature f = k*128 + p.
    h16 = hidden.astype(np.float16)
    h_pack = np.ascontiguousarray(
        h16.reshape(B, KS, 128, H).transpose(0, 2, 1, 3)
    ).reshape(B, 128, KS * H)
    seg_pack = np.zeros((B, 128, KS + 1), np.float32)
    seg_pack[:, :, :KS] = seg_eff.astype(np.float32).reshape(B, KS, 128).transpose(0, 2, 1)
    seg_pack[:, :MAX_SENT, KS] = inv_cnt
    w1_pack = np.ascontiguousarray(
        W1.astype(np.float16).reshape(KH, 128, N1, 512).transpose(1, 2, 0, 3)
    ).reshape(128, N1 * KH * 512)
    w2_pack = np.ascontiguousarray(
        W2.astype(np.float16).reshape(KF1, 128, F2).transpose(1, 0, 2)
    ).reshape(128, KF1 * F2)
    # W3 broadcast across the 64 sentence partitions for the DVE classifier
    w3_pack = np.ascontiguousarray(
        np.broadcast_to(
            W3.T.astype(np.float16).reshape(1, NCLS, F2), (MAX_SENT, NCLS, F2)
        )
    )

    with_b1 = bool(np.any(b1))
    with_b2 = bool(np.any(b2))
    nc = _build(with_b1, with_b2, tuple(float(v) for v in b3))

    in_maps = []
    for c in range(N_CORES):
        m = {
            "h": h_pack[c],
            "seg": seg_pack[c],
            "w1": w1_pack,
            "w2": w2_pack,
            "w3": w3_pack,
        }
        if with_b1:
            m["b1"] = b1.astype(np.float16).reshape(1, F1)
        if with_b2:
            m["b2"] = b2.astype(np.float16).reshape(1, F2)
        in_maps.append(m)

    res = run_bass_kernel_spmd(nc, in_maps, list(range(N_CORES)))
    LAST_META.clear()
    LAST_META["exec_time_ns"] = res.exec_time_ns
    LAST_META["mean_exec_time_ns"] = res.mean_exec_time_ns
    if res.instructions_and_trace is not None:
        LAST_META["trace"] = res.instructions_and_trace[1]

    return np.stack([res.results[c]["out"] for c in range(N_CORES)], axis=0)
